# revision 1
# baseline (speedup 1.0000x reference)
"""NodeAttAggregator Trainium2 kernel.

Math (reference):
    q = hedge @ Wq;  k = node @ Wq
    S = (q @ k.T) / sqrt(FOUT)  masked to member (row,col) pairs, row-softmax
    out = (softmax(S) @ k).T                       # [FOUT, H]

Key identities used here (k is never materialized):
    S[r, c]  = (hedge[r] @ (Wq Wq.T) / sqrt(FOUT)) . node[c]
    out[:,r] = Wq.T @ (sum_c attn[r,c] * node[c])

Each hyperedge has exactly DEG=32 members, so the softmax over N=20000
columns reduces to a softmax over that row's <=32 distinct member columns.
Duplicate (row,col) incidence pairs must count ONCE: the host computes a
-1e30 bias for repeated pairs, which the kernel folds in as the initial
value of each score reduction (exp then gives them exactly 0 weight).

Sharding: rows (hyperedges) split across 8 cores, 512 rows each, processed
as 4 blocks of 128 rows (rows on SBUF partitions). Per block the core
gathers its 128x32 member node rows (4096 x 1KB) from HBM with dma_gather.
node_embed/Wq are replicated; only ~16.8MB of node rows are actually read
per core.
"""

import numpy as np

H, N, FIN, FOUT, DEG = 4096, 20000, 256, 128, 32
NCORES = 8
RPC = H // NCORES            # 512 rows per core
NBLK = RPC // 128            # 4 blocks of 128 rows
IDX_PER_BLK = 128 * DEG      # 4096 gather indices per block
IDXW = IDX_PER_BLK // 16     # 256 wrapped idx columns per block
NJ_DVE = 20                  # member slots aggregated on DVE (rest: ACT+Pool)
SCALE = 1.0 / float(np.sqrt(np.float32(FOUT)))

GATHER_MODE = "indirect"  # "dma_gather" | "indirect" | "memset"

_CACHE = {}


def _build_nc(mode=None):
    import concourse.bacc as bacc
    import concourse.bass as bass
    import concourse.mybir as mybir
    from concourse import masks
    from concourse.tile import TileContext

    if mode is None:
        mode = GATHER_MODE
    f32 = mybir.dt.float32
    i16 = mybir.dt.int16
    i32 = mybir.dt.int32
    Alu = mybir.AluOpType
    Act = mybir.ActivationFunctionType

    nc = bacc.Bacc()
    hedge = nc.declare_dram_parameter("hedge_s", [RPC, FIN], f32, isOutput=False)
    nodes = nc.declare_dram_parameter("node_embed", [N, FIN], f32, isOutput=False)
    wq_in = nc.declare_dram_parameter("wq", [FIN, FOUT], f32, isOutput=False)
    if mode == "dma_gather":
        idx_in = nc.declare_dram_parameter(
            "idx16", [128, NBLK * IDXW], i16, isOutput=False
        )
    else:
        idx_in = nc.declare_dram_parameter(
            "idx32", [128, NBLK * DEG], i32, isOutput=False
        )
    bias_in = nc.declare_dram_parameter(
        "bias_t", [128, NBLK * DEG], f32, isOutput=False
    )
    out_t = nc.declare_dram_parameter("out_t", [FOUT, RPC], f32, isOutput=True)

    with TileContext(nc) as tc:
        with (
            tc.tile_pool(name="const", bufs=1) as constp,
            tc.tile_pool(name="ng", bufs=3) as ngp,
            tc.tile_pool(name="work", bufs=2) as workp,
            tc.tile_pool(name="ps", bufs=2, space="PSUM") as psp,
        ):
            ident = constp.tile([128, 128], f32)
            masks.make_identity(nc, ident[:])

            # ---- constants / setup ------------------------------------
            # Wq as two 128-row chunks: wq_t[:, a, :] = Wq[128a:128a+128, :]
            wq_t = constp.tile([128, 2, FOUT], f32)
            nc.sync.dma_start(wq_t[:, 0, :], wq_in[0:128, :])
            nc.sync.dma_start(wq_t[:, 1, :], wq_in[128:256, :])

            # WqT [fout=128, fin=256]
            wqT = constp.tile([128, FIN], f32)
            for a in range(2):
                ps = psp.tile([128, 128], f32, tag="tr")
                nc.tensor.transpose(ps[:], wq_t[:, a, :], ident[:])
                nc.scalar.copy(wqT[:, a * 128 : (a + 1) * 128], ps[:])

            # G = Wq @ Wq.T [256, 256] as two chunks g[:, a, :] = G[128a:.., :]
            g_t = constp.tile([128, 2, FIN], f32)
            for a in range(2):
                ps = psp.tile([128, FIN], f32, tag="mmg")
                nc.tensor.matmul(
                    ps[:], wqT[:, a * 128 : (a + 1) * 128], wqT[:],
                    start=True, stop=True,
                )
                nc.scalar.copy(g_t[:, a, :], ps[:])

            # hedgeT per block: hT[:, b, a, :] = hedge_s[128b:.., 128a:..].T
            hT = constp.tile([128, NBLK, 2, 128], f32)
            for b in range(NBLK):
                he = workp.tile([128, FIN], f32, tag="hedge")
                nc.sync.dma_start(he[:], hedge[b * 128 : (b + 1) * 128, :])
                for a in range(2):
                    ps = psp.tile([128, 128], f32, tag="tr")
                    nc.tensor.transpose(
                        ps[:], he[:, a * 128 : (a + 1) * 128], ident[:]
                    )
                    nc.scalar.copy(hT[:, b, a, :], ps[:])

            # qW[:, b, :] = (hedge_s @ G) block b   [128 rows, 256]
            qW = constp.tile([128, NBLK, FIN], f32)
            for b in range(NBLK):
                ps = psp.tile([128, FIN], f32, tag="mmg")
                for a in range(2):
                    nc.tensor.matmul(
                        ps[:], hT[:, b, a, :], g_t[:, a, :],
                        start=(a == 0), stop=(a == 1),
                    )
                nc.scalar.copy(qW[:, b, :], ps[:])

            if mode == "dma_gather":
                idxt = constp.tile([128, NBLK * IDXW], i16)
            else:
                idxt = constp.tile([128, NBLK * DEG], i32)
            nc.sync.dma_start(idxt[:], idx_in[:])
            biast = constp.tile([128, NBLK * DEG], f32)
            nc.sync.dma_start(biast[:], bias_in[:])

            S = constp.tile([128, NBLK * DEG], f32)
            Sm = constp.tile([128, NBLK * DEG], f32)
            E = constp.tile([128, NBLK * DEG], f32)
            En = constp.tile([128, NBLK * DEG], f32)
            Z = constp.tile([128, NBLK], f32)
            Zi = constp.tile([128, NBLK], f32)
            junk = constp.tile([128, FIN], f32)

            # ---- per-block pipeline -----------------------------------
            for b in range(NBLK):
                ng = ngp.tile([128, DEG * FIN], f32, tag="ng")
                ng3 = ng[:].rearrange("p (j e) -> p j e", e=FIN)
                if mode == "dma_gather":
                    nc.gpsimd.dma_gather(
                        out_ap=ng3,
                        in_ap=nodes[:],
                        idxs_ap=idxt[:, b * IDXW : (b + 1) * IDXW],
                        num_idxs=IDX_PER_BLK,
                        num_idxs_reg=IDX_PER_BLK,
                        elem_size=FIN,
                    )
                elif mode == "indirect":
                    for j in range(DEG):
                        col = b * DEG + j
                        nc.gpsimd.indirect_dma_start(
                            out=ng3[:, j, :],
                            out_offset=None,
                            in_=nodes[:],
                            in_offset=bass.IndirectOffsetOnAxis(
                                ap=idxt[:, col : col + 1], axis=0
                            ),
                        )
                else:  # memset probe
                    nc.gpsimd.memset(ng[:], 0.03)

                # scores: S[:, b*DEG+j] = sum_f ng[p,j,f] * qW[p,f]
                # (tensor_tensor_reduce crashes the exec unit on this HW;
                #  scalar_tensor_tensor with accum_out is the safe spelling)
                for j in range(DEG):
                    col = b * DEG + j
                    nc.vector.scalar_tensor_tensor(
                        out=junk[:],
                        in0=ng3[:, j, :],
                        scalar=0.0,
                        in1=qW[:, b, :],
                        op0=Alu.bypass,
                        op1=Alu.mult,
                        accum_out=S[:, col : col + 1],
                    )

                # fold in the duplicate-suppression bias, then
                # E = exp(S) (scores are O(10), no max-sub needed); Z = row sum
                bs = slice(b * DEG, (b + 1) * DEG)
                nc.vector.tensor_tensor(
                    out=Sm[:, bs], in0=S[:, bs], in1=biast[:, bs], op=Alu.add
                )
                nc.scalar.activation(
                    out=E[:, bs],
                    in_=Sm[:, bs],
                    func=Act.Exp,
                    accum_out=Z[:, b : b + 1],
                )
                # normalized attention weights En = E / Z (cheap: [128, 32])
                nc.vector.reciprocal(Zi[:, b : b + 1], Z[:, b : b + 1])
                nc.vector.tensor_scalar(
                    out=En[:, bs], in0=E[:, bs],
                    scalar1=Zi[:, b : b + 1], scalar2=None, op0=Alu.mult,
                )

                # acc = sum_j En_j * ng_j, two independent partial chains:
                #  - DVE: fused mult+add (scalar_tensor_tensor) for NJ_DVE js
                #  - ACT (mult, per-partition scale) + Pool (add) for the rest
                # joined for free in the PSUM-accumulated output matmul.
                accD = workp.tile([128, FIN], f32, tag="accD")
                accD2 = workp.tile([128, FIN], f32, tag="accD2")
                accP = workp.tile([128, FIN], f32, tag="accP")
                accP2 = workp.tile([128, FIN], f32, tag="accP2")

                col0 = b * DEG
                nc.vector.tensor_scalar(
                    out=accD[:], in0=ng3[:, 0, :],
                    scalar1=En[:, col0 : col0 + 1], scalar2=None, op0=Alu.mult,
                )
                cur_d, nxt_d = accD, accD2
                for j in range(1, NJ_DVE):
                    col = b * DEG + j
                    nc.vector.scalar_tensor_tensor(
                        out=nxt_d[:], in0=ng3[:, j, :],
                        scalar=En[:, col : col + 1], in1=cur_d[:],
                        op0=Alu.mult, op1=Alu.add,
                    )
                    cur_d, nxt_d = nxt_d, cur_d

                nc.scalar.activation(
                    out=accP[:], in_=ng3[:, NJ_DVE, :], func=Act.Copy,
                    scale=En[:, b * DEG + NJ_DVE : b * DEG + NJ_DVE + 1],
                )
                cur_p, nxt_p = accP, accP2
                for j in range(NJ_DVE + 1, DEG):
                    col = b * DEG + j
                    term = workp.tile([128, FIN], f32, tag="term")
                    nc.scalar.activation(
                        out=term[:], in_=ng3[:, j, :], func=Act.Copy,
                        scale=En[:, col : col + 1],
                    )
                    nc.gpsimd.tensor_tensor(
                        out=nxt_p[:], in0=term[:], in1=cur_p[:], op=Alu.add,
                    )
                    cur_p, nxt_p = nxt_p, cur_p

                # out.T block = Wq.T @ (accD + accP).T -> [fout=128, rows=128]
                aggT = workp.tile([128, 4, 128], f32, tag="aggT")
                for i, acc in enumerate((cur_d, cur_p)):
                    for a in range(2):
                        ps = psp.tile([128, 128], f32, tag="tr")
                        nc.tensor.transpose(
                            ps[:], acc[:, a * 128 : (a + 1) * 128], ident[:]
                        )
                        nc.scalar.copy(aggT[:, 2 * i + a, :], ps[:])
                po = psp.tile([128, 128], f32, tag="mmo")
                for i in range(2):
                    for a in range(2):
                        nc.tensor.matmul(
                            po[:], wq_t[:, a, :], aggT[:, 2 * i + a, :],
                            start=(i == 0 and a == 0), stop=(i == 1 and a == 1),
                        )
                ot = workp.tile([128, 128], f32, tag="ot")
                nc.scalar.copy(ot[:], po[:])
                nc.sync.dma_start(out_t[:, b * 128 : (b + 1) * 128], ot[:])

    nc.finalize()
    return nc


def get_nc():
    key = ("nc", GATHER_MODE)
    if key not in _CACHE:
        _CACHE[key] = _build_nc(GATHER_MODE)
    return _CACHE[key]


def make_in_maps(hedge_embed, node_embed, Wq, row_idx, col_idx):
    """Host-side sharding + index/bias preparation."""
    hedge_embed = np.asarray(hedge_embed, dtype=np.float32)
    node_embed = np.asarray(node_embed, dtype=np.float32)
    Wq = np.asarray(Wq, dtype=np.float32)
    row_idx = np.asarray(row_idx).astype(np.int64)
    col_idx = np.asarray(col_idx).astype(np.int64)

    # Group pairs by row. The reference emits row_idx = repeat(arange(H), DEG);
    # fall back to a stable sort if the layout ever differs.
    expect = np.repeat(np.arange(H, dtype=np.int64), DEG)
    if np.array_equal(row_idx, expect):
        cols = col_idx.reshape(H, DEG)
    else:
        order = np.argsort(row_idx, kind="stable")
        assert np.array_equal(row_idx[order], expect), "rows must have DEG pairs"
        cols = col_idx[order].reshape(H, DEG)

    # Duplicate (row,col) pairs beyond the first get a -1e30 score bias so
    # exp() zeroes them (the reference's member mask counts each col once).
    order = np.argsort(cols, axis=1, kind="stable")
    sc = np.take_along_axis(cols, order, axis=1)
    dup_sorted = np.zeros_like(sc, dtype=bool)
    dup_sorted[:, 1:] = sc[:, 1:] == sc[:, :-1]
    dup = np.zeros((H, DEG), dtype=bool)
    np.put_along_axis(dup, order, dup_sorted, axis=1)
    bias = np.where(dup, np.float32(-1e30), np.float32(0.0))

    hedge_s = hedge_embed * np.float32(SCALE)

    mode = GATHER_MODE
    in_maps = []
    for c in range(NCORES):
        r0 = c * RPC
        ccols = cols[r0 : r0 + RPC]
        cbias = bias[r0 : r0 + RPC]
        bias_t = np.empty((128, NBLK * DEG), np.float32)
        idx16 = np.empty((128, NBLK * IDXW), np.int16)
        idx32 = np.empty((128, NBLK * DEG), np.int32)
        for b in range(NBLK):
            blk = ccols[b * 128 : (b + 1) * 128]          # [128 p, 32 j]
            unw = blk.T.reshape(-1).astype(np.int16)      # i = j*128 + p
            wrapped = unw.reshape(IDXW, 16).T             # [16, 256]
            idx16[:, b * IDXW : (b + 1) * IDXW] = np.tile(wrapped, (8, 1))
            idx32[:, b * DEG : (b + 1) * DEG] = blk
            bias_t[:, b * DEG : (b + 1) * DEG] = cbias[b * 128 : (b + 1) * 128]
        m = {
            "hedge_s": np.ascontiguousarray(hedge_s[r0 : r0 + RPC]),
            "node_embed": node_embed,
            "wq": Wq,
            "bias_t": bias_t,
        }
        if mode == "dma_gather":
            m["idx16"] = idx16
        else:
            m["idx32"] = idx32
        in_maps.append(m)
    return in_maps


def run(in_maps, **kwargs):
    from concourse.bass_utils import run_bass_kernel_spmd

    nc = get_nc()
    return run_bass_kernel_spmd(nc, in_maps, list(range(NCORES)), **kwargs)


def kernel(hedge_embed, node_embed, Wq, row_idx, col_idx):
    in_maps = make_in_maps(hedge_embed, node_embed, Wq, row_idx, col_idx)
    res = run(in_maps)
    out = np.concatenate(
        [res.results[c]["out_t"] for c in range(NCORES)], axis=1
    )
    return np.ascontiguousarray(out.astype(np.float32))

